# revision 20
# baseline (speedup 1.0000x reference)
"""HSTU attention (B=2, L=2048, D=1024, H=16) on 8 TRN2 NeuronCores.

Sharding: heads across cores (2 heads = 128 features per core), both
batches on every core, W_o row-sharded; host sums the 8 partial outputs.

Per batch, tokens are sorted on the host into [valid prompt | valid
items by position | tail], and the WHOLE computation runs in sorted
space (the host unpermutes the output rows).  Keys tile the first
ceil(valid/128)*128 sorted positions with per-key thresholds (0 for
prompt keys, original position for item keys, +inf for tail), so
(key-tile, query-chunk) score tiles classify as skipped (no query in
the chunk reaches the tile's lowest threshold), dense (every query
clears its highest), or staircase (bf16 multiplicative 0/1 mask applied
to exp(S)).  One x load feeds K/V (valid chunks only) and Q/U.

Everything runs in bf16: projections, scores S^T (both heads of a key
tile paired into one [128,1024] psum / one exp), exp on ACT with bf16
output, AV flipped to O[i, dk] with the e-tile as the stationary operand
(65-row matmuls; a ones column in V yields softmax row sums; PSUM
accumulator banks are opened with an explicit zeroing outer product
because psum start zeroes lazily at whole-bank granularity), one fused
scalar_tensor_tensor per gate tile, PE-transpose of the gated output,
and the row-sharded W_o partial projection.  Staircase mask muls ride on
the Pool engine (SBUF-only); psum evictions run on DVE.

All work streams through a single filler queue with per-tile prereq
drains, so exp work starts a few microseconds in and projections fill
PE gaps between score/AV matmuls.
"""

import sys

for _p in ("/opt/trn_rl_repo", "/root/.axon_site/_ro/trn_rl_repo"):
    if _p not in sys.path:
        sys.path.insert(0, _p)

import numpy as np
import ml_dtypes

import concourse.bass as bass  # noqa: F401
import concourse.mybir as mybir
import concourse.tile as tile
from concourse import bacc
from concourse.bass_utils import run_bass_kernel_spmd

F32 = mybir.dt.float32
BF16 = mybir.dt.bfloat16
FP8 = mybir.dt.float8e4
EXP = mybir.ActivationFunctionType.Exp
MUL = mybir.AluOpType.mult
BF = ml_dtypes.bfloat16
F8 = ml_dtypes.float8_e4m3fn

B, L, D, H = 2, 2048, 1024, 16
HPC = 2              # heads per core
E = HPC * 64         # 128 features per core
NDC = D // 128       # 8 contraction chunks
NIC = L // 512       # 4 query chunks of 512
BIG = 10 ** 9

_cache = {}


def _mk_schedule(token_types, seq_lens):
    """Host-side token sort + tile classification. Returns (sig, sched, masks)."""
    jr = np.arange(L)
    sched = []
    masks = []
    for b in range(B):
        tt = np.asarray(token_types[b])
        sl = int(seq_lens[b])
        valid = jr < sl
        pidx = jr[(tt < 3) & valid]
        iidx = jr[(tt >= 3) & valid]
        rest = jr[~valid]
        nv = len(pidx) + len(iidx)
        nk = -(-nv // 128)
        perm = np.concatenate([pidx, iidx, rest])
        thresh = np.full(nk * 128, BIG, np.int64)
        thresh[: len(pidx)] = 0
        thresh[len(pidx) : nv] = iidx
        qpos = perm.astype(np.int64)
        live = [[] for _ in range(NIC)]   # (t, lo_col, mask_idx|None, slist)
        for c in range(NIC):
            q = qpos[512 * c : 512 * (c + 1)]
            for t in range(nk):
                th = thresh[128 * t : 128 * (t + 1)]
                t_lo, t_hi = int(th[0]), int(th[-1])
                ok = q >= t_lo
                if not ok.any():
                    continue
                lo = int(np.argmax(ok)) // 128 * 128
                slist = [s for s in range(4) if ok[128 * s : 128 * (s + 1)].any()]
                if (q >= t_hi).all():
                    live[c].append((t, lo, None, slist))
                else:
                    masks.append((q[None, :] >= th[:, None]).astype(np.float32))
                    live[c].append((t, lo, len(masks) - 1, slist))
        s_live = [
            sorted({s for (_t, _lo, _mi, sl_) in live[c] for s in sl_})
            for c in range(NIC)
        ]
        kchunks = []
        off = 0
        while off < nk * 128:
            sz = min(512, nk * 128 - off)
            kchunks.append((off, sz))
            off += sz
        sched.append(
            dict(nk=nk, nv=nv, perm=perm, live=live, s_live=s_live, kchunks=kchunks)
        )
    nm = max(1, len(masks))
    masks_np = np.zeros((128, nm, 512), F8)
    for i, m in enumerate(masks):
        masks_np[:, i, :] = m.astype(F8)
    sig = tuple(
        (
            s["nk"],
            tuple(
                tuple(
                    (t, lo, mi is not None, tuple(sl_))
                    for (t, lo, mi, sl_) in s["live"][c]
                )
                for c in range(NIC)
            ),
        )
        for s in sched
    )
    return sig, sched, masks_np


def _build(sched, nm):
    nc = bacc.Bacc("TRN2", target_bir_lowering=False, debug=False)

    nk = [sched[b]["nk"] for b in range(B)]
    xd = [
        nc.dram_tensor(f"x{b}", [NIC, 128, NDC, 512], BF16, kind="ExternalInput").ap()
        for b in range(B)
    ]
    wd = {
        w: nc.dram_tensor(w, [128, NDC, E], BF16, kind="ExternalInput").ap()
        for w in ("wq", "wk", "wv", "wu")
    }
    wod = nc.dram_tensor("wo", [128, D], BF16, kind="ExternalInput").ap()
    idd = nc.dram_tensor("ident", [128, 128], BF16, kind="ExternalInput").ap()
    mkd = nc.dram_tensor("masks", [128, nm, 512], FP8, kind="ExternalInput").ap()
    outd = nc.dram_tensor("outp", [B, NIC, 128, 4, D], BF16, kind="ExternalOutput").ap()

    with tile.TileContext(nc) as tc:
        with tc.tile_pool(name="persist", bufs=1) as pp:
            kt = [pp.tile([128, nk[b] * 128], BF16, tag=f"kt{b}", name=f"kt{b}") for b in range(B)]
            qt = [pp.tile([128, L], BF16, tag=f"qt{b}", name=f"qt{b}") for b in range(B)]
            vt = [
                [pp.tile([128, nk[b], 65], BF16, tag=f"v{b}{h}", name=f"v{b}{h}") for h in range(HPC)]
                for b in range(B)
            ]
            ut = [pp.tile([128, 16, E], BF16, tag=f"u{b}", name=f"u{b}") for b in range(B)]
            wt = {
                w: pp.tile([128, NDC, E], BF16, tag=w, name=f"{w}t")
                for w in ("wq", "wk", "wv", "wu")
            }
            wot = pp.tile([128, D], BF16, tag="wo", name="wot")
            idt = pp.tile([128, 128], BF16, tag="id", name="idt")
            mkt = pp.tile([128, nm, 512], FP8, tag="mk", name="mkt")
            zt = pp.tile([1, 512], BF16, tag="zt", name="zt")
            nc.vector.memset(zt, 0.0)

            nc.sync.dma_start(out=wt["wk"], in_=wd["wk"])
            for b in range(B):
                for h in range(HPC):
                    nc.vector.memset(vt[b][h][:, :, 64:65], 1.0)

            with tc.tile_pool(name="xs", bufs=6) as xs, \
                 tc.tile_pool(name="epool", bufs=6) as ep, \
                 tc.tile_pool(name="gpool", bufs=2) as gp, \
                 tc.tile_pool(name="rpool", bufs=2) as rp, \
                 tc.tile_pool(name="stage", bufs=2) as stp, \
                 tc.tile_pool(name="ps_sp", bufs=2, space="PSUM") as ps_sp, \
                 tc.tile_pool(name="ps_av", bufs=1, space="PSUM") as ps_av, \
                 tc.tile_pool(name="ps_m1", bufs=1, space="PSUM") as ps_m1, \
                 tc.tile_pool(name="ps_m2", bufs=1, space="PSUM") as ps_m2:

                misc_state = [0]

                def misc_pool():
                    misc_state[0] ^= 1
                    return ps_m1 if misc_state[0] else ps_m2

                chunk_tiles = {}
                chain_ps = {}

                def load_chunk(b, c):
                    t = xs.tile([128, NDC, 512], BF16, tag="xc", name="xc")
                    nc.sync.dma_start(out=t, in_=xd[b][c])
                    chunk_tiles[(b, c)] = t

                # ---------------- projection work items ----------------
                def emit_k(b, ci, half):
                    off, sz = sched[b]["kchunks"][ci]
                    xc = chunk_tiles[(b, ci)]
                    if half == 0:
                        chain_ps[("k", b, ci)] = misc_pool().tile(
                            [128, 512], F32, tag="m", name="kp"
                        )
                    p = chain_ps[("k", b, ci)]
                    for dc in range(4 * half, 4 * half + 4):
                        nc.tensor.matmul(
                            p[:, 0:sz], wt["wk"][:, dc, :], xc[:, dc, 0:sz],
                            start=(dc == 0), stop=(dc == NDC - 1),
                        )
                    if half == 1:
                        with nc.allow_low_precision(reason="bf16 K"):
                            nc.vector.tensor_copy(kt[b][:, off : off + sz], p[:, 0:sz])

                def emit_v(b, ci, tl):
                    xc = chunk_tiles[(b, ci)]
                    tg = 4 * ci + tl
                    pv = misc_pool().tile([128, 512], F32, tag="m", name="vp")
                    for dc in range(NDC):
                        nc.tensor.matmul(
                            pv[:, 0:E],
                            xc[:, dc, 128 * tl : 128 * (tl + 1)],
                            wt["wv"][:, dc, :],
                            start=(dc == 0), stop=(dc == NDC - 1),
                        )
                    with nc.allow_low_precision(reason="bf16 V"):
                        nc.vector.tensor_copy(vt[b][0][:, tg, 0:64], pv[:, 0:64])
                        nc.vector.tensor_copy(vt[b][1][:, tg, 0:64], pv[:, 64:128])

                def emit_q(b, c, half):
                    xc = chunk_tiles[(b, c)]
                    if half == 0:
                        chain_ps[("q", b, c)] = misc_pool().tile(
                            [128, 512], F32, tag="m", name="qp"
                        )
                    p = chain_ps[("q", b, c)]
                    for dc in range(4 * half, 4 * half + 4):
                        nc.tensor.matmul(
                            p, wt["wq"][:, dc, :], xc[:, dc, :],
                            start=(dc == 0), stop=(dc == NDC - 1),
                        )
                    if half == 1:
                        with nc.allow_low_precision(reason="bf16 Q"):
                            nc.vector.tensor_copy(qt[b][:, 512 * c : 512 * (c + 1)], p)

                def emit_u(b, c, k):
                    xc = chunk_tiles[(b, c)]
                    pu = misc_pool().tile([128, 512], F32, tag="m", name="up")
                    for dc in range(NDC):
                        nc.tensor.matmul(
                            pu[:, 0:E],
                            xc[:, dc, 128 * k : 128 * (k + 1)],
                            wt["wu"][:, dc, :],
                            start=(dc == 0), stop=(dc == NDC - 1),
                        )
                    with nc.allow_low_precision(reason="bf16 U"):
                        nc.vector.tensor_copy(ut[b][:, 4 * c + k, :], pu[:, 0:E])

                # ---------------- W_o work items ----------------
                def emit_wo(b, c, g, stg, k):
                    gtp = misc_pool().tile([128, 128], BF16, tag="m", name="gtp")
                    nc.tensor.transpose(gtp, g, idt)
                    gts = gp.tile([128, 128], BF16, tag="gts", name="gts")
                    nc.vector.tensor_copy(gts, gtp)
                    for fc in range(2):
                        wp = misc_pool().tile([128, 512], F32, tag="m", name="wp")
                        nc.tensor.matmul(
                            wp, gts, wot[:, 512 * fc : 512 * (fc + 1)],
                            start=True, stop=True,
                        )
                        with nc.allow_low_precision(reason="bf16 out"):
                            nc.vector.tensor_copy(
                                stg[:, k, 512 * fc : 512 * (fc + 1)], wp
                            )
                    nc.sync.dma_start(out=outd[b][c][:, k, :], in_=stg[:, k, :])

                # ---------------- filler queue ----------------
                filler_q = []          # (group, fn) in dependency-safe order
                remaining = {}

                def push(group, fn):
                    filler_q.append((group, fn))
                    remaining[group] = remaining.get(group, 0) + 1

                def push_front(group, fn):
                    filler_q.insert(0, (group, fn))
                    remaining[group] = remaining.get(group, 0) + 1

                def pop_one():
                    if filler_q:
                        group, fn = filler_q.pop(0)
                        remaining[group] -= 1
                        fn()

                def drain(group):
                    while remaining.get(group, 0) > 0:
                        pop_one()

                # ---------------- attention ----------------
                tiles_left = [
                    sum(len(sched[bb]["live"][cc]) for bb in range(B) for cc in range(NIC))
                ]

                def attention(b, c):
                    sb = sched[b]
                    liv = sb["live"][c]
                    nkc = len(sb["kchunks"])
                    av = [
                        ps_av.tile([128, 4, 128], F32, tag=f"av{h}", name=f"av{h}")
                        for h in range(HPC)
                    ]
                    t_last, s_last = liv[-1][0], liv[-1][3][-1]
                    zeroed = [False]

                    def zero_av():
                        # deferred so the next chunk's score matmuls are not
                        # stuck in the PE stream behind gating of the previous
                        # chunk (the zeroing matmul waits on its DVE reads)
                        for h in range(HPC):
                            nc.tensor.matmul(
                                av[h][:, :, :],
                                zt[0:1, 0:128], zt[0:1, :],
                                start=True, stop=False, skip_group_check=True,
                            )
                        zeroed[0] = True
                    prev = None

                    def do_av(t, lo, slist, et):
                        for s in slist:
                            for h in range(HPC):
                                nc.tensor.matmul(
                                    av[h][:, s, 0:65],
                                    et[:, 512 * h + 128 * s : 512 * h + 128 * (s + 1)],
                                    vt[b][h][:, t, :],
                                    start=False,
                                    stop=(t == t_last and s == s_last),
                                    skip_group_check=True,
                                )

                    for t, lo, mi, slist in liv:
                        drain(("p", b, min(t // 4 + 1, nkc - 1)))
                        sp = ps_sp.tile([128, 1024], F32, tag="sp", name="sp")
                        et = ep.tile([128, 1024], BF16, tag="e", name="et")
                        for h in range(HPC):
                            nc.tensor.matmul(
                                sp[:, 512 * h + lo : 512 * h + 512],
                                kt[b][64 * h : 64 * h + 64, 128 * t : 128 * (t + 1)],
                                qt[b][64 * h : 64 * h + 64, 512 * c + lo : 512 * (c + 1)],
                                start=True, stop=True,
                            )
                        if lo == 0:
                            nc.scalar.activation(et, sp, EXP)
                        else:
                            nc.scalar.activation(et[:, lo:512], sp[:, lo:512], EXP)
                            nc.scalar.activation(
                                et[:, 512 + lo : 1024], sp[:, 512 + lo : 1024], EXP
                            )
                        if mi is not None:
                            for h in range(HPC):
                                a0 = 512 * h + lo
                                nc.gpsimd.tensor_tensor(
                                    et[:, a0 : 512 * h + 512],
                                    et[:, a0 : 512 * h + 512],
                                    mkt[:, mi, lo:512],
                                    MUL,
                                )
                        if prev is not None:
                            if not zeroed[0]:
                                zero_av()
                            do_av(*prev)
                        prev = (t, lo, slist, et)
                        npop = 1 + (len(filler_q) > tiles_left[0])
                        for _ in range(npop):
                            pop_one()
                        tiles_left[0] -= 1
                    if not zeroed[0]:
                        zero_av()
                    do_av(*prev)
                    # gating: g = (AV * 1/rowsum) * U, one fused op per (h, s)
                    rec = rp.tile([128, 4, HPC], F32, tag="rec", name="rec")
                    for h in range(HPC):
                        nc.vector.reciprocal(rec[:, :, h : h + 1], av[h][:, :, 64:65])
                    gs = []
                    for s in range(4):
                        g = gp.tile([128, 128], BF16, tag="g", name="g", bufs=8)
                        for h in range(HPC):
                            if s not in sb["s_live"][c]:
                                nc.vector.memset(g[:, 64 * h : 64 * h + 64], 0.0)
                                continue
                            with nc.allow_low_precision(reason="bf16 gate"):
                                nc.vector.scalar_tensor_tensor(
                                    g[:, 64 * h : 64 * h + 64],
                                    av[h][:, s, 0:64],
                                    rec[:, s, h : h + 1],
                                    ut[b][:, 4 * c + s, 64 * h : 64 * h + 64],
                                    MUL,
                                    MUL,
                                )
                        gs.append(g)
                    return gs

                # ---------------- top-level schedule ----------------
                def push_chunk(b, c):
                    g_p = ("p", b, c)
                    if b > 0:
                        push(g_p, lambda: load_chunk(b, c))
                    if c < len(sched[b]["kchunks"]):
                        _off, sz = sched[b]["kchunks"][c]
                        push(g_p, lambda: emit_k(b, c, 0))
                        push(g_p, lambda: emit_k(b, c, 1))
                        for tl in range(sz // 128):
                            push(g_p, lambda tl=tl: emit_v(b, c, tl))
                    g_q = (b, c)
                    push(g_q, lambda: emit_q(b, c, 0))
                    push(g_q, lambda: emit_q(b, c, 1))
                    for k in range(4):
                        push(g_q, lambda k=k: emit_u(b, c, k))

                # front-loaded DMAs: first x chunk, weights+masks, rest of b0
                load_chunk(0, 0)
                for w in ("wv", "wq", "wu"):
                    nc.sync.dma_start(out=wt[w], in_=wd[w])
                nc.sync.dma_start(out=mkt, in_=mkd)
                load_chunk(0, 1)
                nc.sync.dma_start(out=wot, in_=wod)
                nc.sync.dma_start(out=idt, in_=idd)
                load_chunk(0, 2)
                load_chunk(0, 3)
                for b in range(B):
                    for c in range(NIC):
                        push_chunk(b, c)

                for b in range(B):
                    for c in range(NIC):
                        drain((b, c))
                        gs = attention(b, c)
                        stg = stp.tile([128, 4, D], BF16, tag="st", name="stg")
                        for k in range(3, -1, -1):
                            push_front(
                                ("wo", b, c),
                                lambda b=b, c=c, g=gs[k], stg=stg, k=k: emit_wo(
                                    b, c, g, stg, k
                                ),
                            )
                while filler_q:
                    pop_one()

    nc.compile()
    return nc


def _host_inputs(x, token_types, seq_lens, W_q, W_k, W_v, W_u, W_o, sched, masks_np):
    x = np.asarray(x, dtype=np.float32)
    W = {
        "wq": np.asarray(W_q, np.float32) / 8.0,
        "wk": np.asarray(W_k, np.float32),
        "wv": np.asarray(W_v, np.float32),
        "wu": np.asarray(W_u, np.float32),
    }
    Wo = np.asarray(W_o, np.float32)
    shared = {"ident": np.eye(128, dtype=BF), "masks": masks_np}
    for b in range(B):
        xb = x[b].astype(BF)[sched[b]["perm"]]  # sorted tokens [L, D]
        shared[f"x{b}"] = np.ascontiguousarray(
            xb.reshape(NIC, 512, NDC, 128).transpose(0, 3, 2, 1)
        )  # [c, p, dc, l']
    in_maps = []
    for core in range(8):
        e0 = E * core
        im = dict(shared)
        for w, Wm in W.items():
            im[w] = np.ascontiguousarray(
                Wm[e0 : e0 + E].astype(BF).reshape(E, NDC, 128).transpose(2, 1, 0)
            )  # [p, dc, e]
        im["wo"] = np.ascontiguousarray(Wo[:, e0 : e0 + E].astype(BF).T)  # [p, d]
        in_maps.append(im)
    return in_maps


def kernel(x, token_types, seq_lens, W_q, W_k, W_v, W_u, W_o, **_run_kwargs):
    sig, sched, masks_np = _mk_schedule(np.asarray(token_types), np.asarray(seq_lens))
    if _cache.get("sig") != sig:
        _cache["nc"] = _build(sched, masks_np.shape[1])
        _cache["sig"] = sig
    nc = _cache["nc"]
    in_maps = _host_inputs(
        x, token_types, seq_lens, W_q, W_k, W_v, W_u, W_o, sched, masks_np
    )
    try:
        res = run_bass_kernel_spmd(nc, in_maps, list(range(8)), **_run_kwargs)
    except Exception as ex:
        if "UNRECOVERABLE" not in str(ex) and "UNAVAILABLE" not in str(ex):
            raise
        res = run_bass_kernel_spmd(nc, in_maps, list(range(8)), **_run_kwargs)
    _cache["last_result"] = res
    full = np.zeros((B, L, D), np.float64)
    for core in range(8):
        o = res.results[core]["outp"].astype(np.float64)  # [b, c, p, k, d]
        full += o.transpose(0, 1, 3, 2, 4).reshape(B, L, D)
    out = np.empty_like(full)
    for b in range(B):
        out[b, sched[b]["perm"]] = full[b]
    return out.astype(np.float32)


# revision 21
# speedup vs baseline: 1.0555x; 1.0555x over previous
"""HSTU attention (B=2, L=2048, D=1024, H=16) on 8 TRN2 NeuronCores.

Sharding: heads across cores (2 heads = 128 features per core), both
batches on every core, W_o row-sharded; host sums the 8 partial outputs.

Per batch, tokens are sorted on the host into [valid prompt | valid
items by position | tail], and the WHOLE computation runs in sorted
space (the host unpermutes the output rows).  Keys tile the first
ceil(valid/128)*128 sorted positions with per-key thresholds (0 for
prompt keys, original position for item keys, +inf for tail), so
(key-tile, query-chunk) score tiles classify as skipped (no query in
the chunk reaches the tile's lowest threshold), dense (every query
clears its highest), or staircase (bf16 multiplicative 0/1 mask applied
to exp(S)).  One x load feeds K/V (valid chunks only) and Q/U.

Everything runs in bf16: projections, scores S^T (both heads of a key
tile paired into one [128,1024] psum / one exp), exp on ACT with bf16
output, AV flipped to O[i, dk] with the e-tile as the stationary operand
(65-row matmuls; a ones column in V yields softmax row sums; PSUM
accumulator banks are opened with an explicit zeroing outer product
because psum start zeroes lazily at whole-bank granularity), one fused
scalar_tensor_tensor per gate tile, PE-transpose of the gated output,
and the row-sharded W_o partial projection.  Staircase mask muls ride on
the Pool engine (SBUF-only); psum evictions run on DVE.

All work streams through a single filler queue with per-tile prereq
drains, so exp work starts a few microseconds in and projections fill
PE gaps between score/AV matmuls.
"""

import sys

for _p in ("/opt/trn_rl_repo", "/root/.axon_site/_ro/trn_rl_repo"):
    if _p not in sys.path:
        sys.path.insert(0, _p)

import numpy as np
import ml_dtypes

import concourse.bass as bass  # noqa: F401
import concourse.mybir as mybir
import concourse.tile as tile
from concourse import bacc
from concourse.bass_utils import run_bass_kernel_spmd

F32 = mybir.dt.float32
BF16 = mybir.dt.bfloat16
FP8 = mybir.dt.float8e4
EXP = mybir.ActivationFunctionType.Exp
MUL = mybir.AluOpType.mult
BF = ml_dtypes.bfloat16
F8 = ml_dtypes.float8_e4m3fn

B, L, D, H = 2, 2048, 1024, 16
HPC = 2              # heads per core
E = HPC * 64         # 128 features per core
NDC = D // 128       # 8 contraction chunks
NIC = L // 512       # 4 query chunks of 512
BIG = 10 ** 9

_cache = {}


def _mk_schedule(token_types, seq_lens):
    """Host-side token sort + tile classification. Returns (sig, sched, masks)."""
    jr = np.arange(L)
    sched = []
    masks = []
    for b in range(B):
        tt = np.asarray(token_types[b])
        sl = int(seq_lens[b])
        valid = jr < sl
        pidx = jr[(tt < 3) & valid]
        iidx = jr[(tt >= 3) & valid]
        rest = jr[~valid]
        nv = len(pidx) + len(iidx)
        nk = -(-nv // 128)
        perm = np.concatenate([pidx, iidx, rest])
        thresh = np.full(nk * 128, BIG, np.int64)
        thresh[: len(pidx)] = 0
        thresh[len(pidx) : nv] = iidx
        qpos = perm.astype(np.int64)
        live = [[] for _ in range(NIC)]   # (t, lo_col, mask_idx|None, slist)
        for c in range(NIC):
            q = qpos[512 * c : 512 * (c + 1)]
            for t in range(nk):
                th = thresh[128 * t : 128 * (t + 1)]
                t_lo, t_hi = int(th[0]), int(th[-1])
                ok = q >= t_lo
                if not ok.any():
                    continue
                lo = int(np.argmax(ok)) // 128 * 128
                slist = [s for s in range(4) if ok[128 * s : 128 * (s + 1)].any()]
                allok = q >= t_hi
                if allok.all():
                    live[c].append((t, lo, None, slist, 512))
                else:
                    # hc: columns >= hc are fully allowed -> no mask needed there
                    hc = 512 - int(np.argmax(~allok[::-1]))
                    masks.append((q[None, :] >= th[:, None]).astype(np.float32))
                    live[c].append((t, lo, len(masks) - 1, slist, hc))
        s_live = [
            sorted({s for (_t, _lo, _mi, sl_, _hc) in live[c] for s in sl_})
            for c in range(NIC)
        ]
        kchunks = []
        off = 0
        while off < nk * 128:
            sz = min(512, nk * 128 - off)
            kchunks.append((off, sz))
            off += sz
        sched.append(
            dict(nk=nk, nv=nv, perm=perm, live=live, s_live=s_live, kchunks=kchunks)
        )
    nm = max(1, len(masks))
    masks_np = np.zeros((128, nm, 512), F8)
    for i, m in enumerate(masks):
        masks_np[:, i, :] = m.astype(F8)
    sig = tuple(
        (
            s["nk"],
            tuple(
                tuple(
                    (t, lo, mi is not None, tuple(sl_), hc)
                    for (t, lo, mi, sl_, hc) in s["live"][c]
                )
                for c in range(NIC)
            ),
        )
        for s in sched
    )
    return sig, sched, masks_np


def _build(sched, nm):
    nc = bacc.Bacc("TRN2", target_bir_lowering=False, debug=False)

    nk = [sched[b]["nk"] for b in range(B)]
    xd = [
        nc.dram_tensor(f"x{b}", [NIC, 128, NDC, 512], BF16, kind="ExternalInput").ap()
        for b in range(B)
    ]
    wd = {
        w: nc.dram_tensor(w, [128, NDC, E], BF16, kind="ExternalInput").ap()
        for w in ("wq", "wk", "wv", "wu")
    }
    wod = nc.dram_tensor("wo", [128, D], BF16, kind="ExternalInput").ap()
    idd = nc.dram_tensor("ident", [128, 128], BF16, kind="ExternalInput").ap()
    mkd = nc.dram_tensor("masks", [128, nm, 512], FP8, kind="ExternalInput").ap()
    outd = nc.dram_tensor("outp", [B, NIC, 128, 4, D], BF16, kind="ExternalOutput").ap()

    with tile.TileContext(nc) as tc:
        with tc.tile_pool(name="persist", bufs=1) as pp:
            kt = [pp.tile([128, nk[b] * 128], BF16, tag=f"kt{b}", name=f"kt{b}") for b in range(B)]
            qt = [pp.tile([128, L], BF16, tag=f"qt{b}", name=f"qt{b}") for b in range(B)]
            vt = [
                [pp.tile([128, nk[b], 65], BF16, tag=f"v{b}{h}", name=f"v{b}{h}") for h in range(HPC)]
                for b in range(B)
            ]
            ut = [pp.tile([128, 16, E], BF16, tag=f"u{b}", name=f"u{b}") for b in range(B)]
            wt = {
                w: pp.tile([128, NDC, E], BF16, tag=w, name=f"{w}t")
                for w in ("wq", "wk", "wv", "wu")
            }
            wot = pp.tile([128, D], BF16, tag="wo", name="wot")
            idt = pp.tile([128, 128], BF16, tag="id", name="idt")
            mkt = pp.tile([128, nm, 512], FP8, tag="mk", name="mkt")
            zt = pp.tile([1, 512], BF16, tag="zt", name="zt")
            nc.vector.memset(zt, 0.0)

            nc.sync.dma_start(out=wt["wk"], in_=wd["wk"])
            for b in range(B):
                for h in range(HPC):
                    nc.vector.memset(vt[b][h][:, :, 64:65], 1.0)

            with tc.tile_pool(name="xs", bufs=6) as xs, \
                 tc.tile_pool(name="epool", bufs=8) as ep, \
                 tc.tile_pool(name="gpool", bufs=2) as gp, \
                 tc.tile_pool(name="rpool", bufs=2) as rp, \
                 tc.tile_pool(name="stage", bufs=2) as stp, \
                 tc.tile_pool(name="ps_sp", bufs=2, space="PSUM") as ps_sp, \
                 tc.tile_pool(name="ps_av", bufs=1, space="PSUM") as ps_av, \
                 tc.tile_pool(name="ps_m1", bufs=1, space="PSUM") as ps_m1, \
                 tc.tile_pool(name="ps_m2", bufs=1, space="PSUM") as ps_m2:

                misc_state = [0]

                def misc_pool():
                    misc_state[0] ^= 1
                    return ps_m1 if misc_state[0] else ps_m2

                chunk_tiles = {}
                chain_ps = {}

                def load_chunk(b, c):
                    t = xs.tile([128, NDC, 512], BF16, tag="xc", name="xc")
                    nc.sync.dma_start(out=t, in_=xd[b][c])
                    chunk_tiles[(b, c)] = t

                # ---------------- projection work items ----------------
                def emit_k(b, ci, half):
                    off, sz = sched[b]["kchunks"][ci]
                    xc = chunk_tiles[(b, ci)]
                    if half == 0:
                        chain_ps[("k", b, ci)] = misc_pool().tile(
                            [128, 512], F32, tag="m", name="kp"
                        )
                    p = chain_ps[("k", b, ci)]
                    for dc in range(4 * half, 4 * half + 4):
                        nc.tensor.matmul(
                            p[:, 0:sz], wt["wk"][:, dc, :], xc[:, dc, 0:sz],
                            start=(dc == 0), stop=(dc == NDC - 1),
                        )
                    if half == 1:
                        with nc.allow_low_precision(reason="bf16 K"):
                            nc.vector.tensor_copy(kt[b][:, off : off + sz], p[:, 0:sz])

                def emit_v(b, ci, tl):
                    xc = chunk_tiles[(b, ci)]
                    tg = 4 * ci + tl
                    pv = misc_pool().tile([128, 512], F32, tag="m", name="vp")
                    for dc in range(NDC):
                        nc.tensor.matmul(
                            pv[:, 0:E],
                            xc[:, dc, 128 * tl : 128 * (tl + 1)],
                            wt["wv"][:, dc, :],
                            start=(dc == 0), stop=(dc == NDC - 1),
                        )
                    with nc.allow_low_precision(reason="bf16 V"):
                        nc.vector.tensor_copy(vt[b][0][:, tg, 0:64], pv[:, 0:64])
                        nc.vector.tensor_copy(vt[b][1][:, tg, 0:64], pv[:, 64:128])

                def emit_q(b, c, half):
                    xc = chunk_tiles[(b, c)]
                    if half == 0:
                        chain_ps[("q", b, c)] = misc_pool().tile(
                            [128, 512], F32, tag="m", name="qp"
                        )
                    p = chain_ps[("q", b, c)]
                    for dc in range(4 * half, 4 * half + 4):
                        nc.tensor.matmul(
                            p, wt["wq"][:, dc, :], xc[:, dc, :],
                            start=(dc == 0), stop=(dc == NDC - 1),
                        )
                    if half == 1:
                        with nc.allow_low_precision(reason="bf16 Q"):
                            nc.vector.tensor_copy(qt[b][:, 512 * c : 512 * (c + 1)], p)

                def emit_u(b, c, k):
                    xc = chunk_tiles[(b, c)]
                    pu = misc_pool().tile([128, 512], F32, tag="m", name="up")
                    for dc in range(NDC):
                        nc.tensor.matmul(
                            pu[:, 0:E],
                            xc[:, dc, 128 * k : 128 * (k + 1)],
                            wt["wu"][:, dc, :],
                            start=(dc == 0), stop=(dc == NDC - 1),
                        )
                    with nc.allow_low_precision(reason="bf16 U"):
                        nc.vector.tensor_copy(ut[b][:, 4 * c + k, :], pu[:, 0:E])

                # ---------------- W_o work items ----------------
                def emit_wo(b, c, g, stg, k):
                    gtp = misc_pool().tile([128, 128], BF16, tag="m", name="gtp")
                    nc.tensor.transpose(gtp, g, idt)
                    gts = gp.tile([128, 128], BF16, tag="gts", name="gts")
                    nc.vector.tensor_copy(gts, gtp)
                    for fc in range(2):
                        wp = misc_pool().tile([128, 512], F32, tag="m", name="wp")
                        nc.tensor.matmul(
                            wp, gts, wot[:, 512 * fc : 512 * (fc + 1)],
                            start=True, stop=True,
                        )
                        with nc.allow_low_precision(reason="bf16 out"):
                            nc.vector.tensor_copy(
                                stg[:, k, 512 * fc : 512 * (fc + 1)], wp
                            )
                    nc.sync.dma_start(out=outd[b][c][:, k, :], in_=stg[:, k, :])

                # ---------------- filler queue ----------------
                filler_q = []          # (group, fn) in dependency-safe order
                remaining = {}

                def push(group, fn):
                    filler_q.append((group, fn))
                    remaining[group] = remaining.get(group, 0) + 1

                def push_front(group, fn):
                    filler_q.insert(0, (group, fn))
                    remaining[group] = remaining.get(group, 0) + 1

                def pop_one():
                    if filler_q:
                        group, fn = filler_q.pop(0)
                        remaining[group] -= 1
                        fn()

                def drain(group):
                    while remaining.get(group, 0) > 0:
                        pop_one()

                # ---------------- attention ----------------
                tiles_left = [
                    sum(len(sched[bb]["live"][cc]) for bb in range(B) for cc in range(NIC))
                ]

                def attention(b, c):
                    sb = sched[b]
                    liv = sb["live"][c]
                    nkc = len(sb["kchunks"])
                    av = [
                        ps_av.tile([128, 4, 128], F32, tag=f"av{h}", name=f"av{h}")
                        for h in range(HPC)
                    ]
                    t_last, s_last = liv[-1][0], liv[-1][3][-1]
                    zeroed = [False]

                    def zero_av():
                        # deferred so the next chunk's score matmuls are not
                        # stuck in the PE stream behind gating of the previous
                        # chunk (the zeroing matmul waits on its DVE reads)
                        for h in range(HPC):
                            nc.tensor.matmul(
                                av[h][:, :, :],
                                zt[0:1, 0:128], zt[0:1, :],
                                start=True, stop=False, skip_group_check=True,
                            )
                        zeroed[0] = True
                    prev = None

                    def do_av(t, lo, slist, et):
                        for s in slist:
                            for h in range(HPC):
                                nc.tensor.matmul(
                                    av[h][:, s, 0:65],
                                    et[:, 512 * h + 128 * s : 512 * h + 128 * (s + 1)],
                                    vt[b][h][:, t, :],
                                    start=False,
                                    stop=(t == t_last and s == s_last),
                                    skip_group_check=True,
                                )

                    for t, lo, mi, slist, hc in liv:
                        drain(("p", b, min(t // 4 + 1, nkc - 1)))
                        sp = ps_sp.tile([128, 1024], F32, tag="sp", name="sp")
                        et = ep.tile([128, 1024], BF16, tag="e", name="et")
                        for h in range(HPC):
                            nc.tensor.matmul(
                                sp[:, 512 * h + lo : 512 * h + 512],
                                kt[b][64 * h : 64 * h + 64, 128 * t : 128 * (t + 1)],
                                qt[b][64 * h : 64 * h + 64, 512 * c + lo : 512 * (c + 1)],
                                start=True, stop=True,
                            )
                        if lo == 0:
                            nc.scalar.activation(et, sp, EXP)
                        else:
                            nc.scalar.activation(et[:, lo:512], sp[:, lo:512], EXP)
                            nc.scalar.activation(
                                et[:, 512 + lo : 1024], sp[:, 512 + lo : 1024], EXP
                            )
                        if mi is not None:
                            for h in range(HPC):
                                a0 = 512 * h + lo
                                a1 = 512 * h + hc
                                nc.gpsimd.tensor_tensor(
                                    et[:, a0:a1],
                                    et[:, a0:a1],
                                    mkt[:, mi, lo:hc],
                                    MUL,
                                )
                        if prev is not None:
                            if not zeroed[0]:
                                zero_av()
                            do_av(*prev)
                        prev = (t, lo, slist, et)
                        npop = 1 + (len(filler_q) > tiles_left[0])
                        for _ in range(npop):
                            pop_one()
                        tiles_left[0] -= 1
                    if not zeroed[0]:
                        zero_av()
                    do_av(*prev)
                    # gating: g = (AV * 1/rowsum) * U, one fused op per (h, s)
                    rec = rp.tile([128, 4, HPC], F32, tag="rec", name="rec")
                    for h in range(HPC):
                        nc.vector.reciprocal(rec[:, :, h : h + 1], av[h][:, :, 64:65])
                    gs = []
                    for s in range(4):
                        g = gp.tile([128, 128], BF16, tag="g", name="g", bufs=8)
                        for h in range(HPC):
                            if s not in sb["s_live"][c]:
                                nc.vector.memset(g[:, 64 * h : 64 * h + 64], 0.0)
                                continue
                            with nc.allow_low_precision(reason="bf16 gate"):
                                nc.vector.scalar_tensor_tensor(
                                    g[:, 64 * h : 64 * h + 64],
                                    av[h][:, s, 0:64],
                                    rec[:, s, h : h + 1],
                                    ut[b][:, 4 * c + s, 64 * h : 64 * h + 64],
                                    MUL,
                                    MUL,
                                )
                        gs.append(g)
                    return gs

                # ---------------- top-level schedule ----------------
                def push_chunk(b, c):
                    g_p = ("p", b, c)
                    if b > 0:
                        push(g_p, lambda: load_chunk(b, c))
                    if c < len(sched[b]["kchunks"]):
                        _off, sz = sched[b]["kchunks"][c]
                        push(g_p, lambda: emit_k(b, c, 0))
                        push(g_p, lambda: emit_k(b, c, 1))
                        for tl in range(sz // 128):
                            push(g_p, lambda tl=tl: emit_v(b, c, tl))
                    g_q = (b, c)
                    push(g_q, lambda: emit_q(b, c, 0))
                    push(g_q, lambda: emit_q(b, c, 1))
                    for k in range(4):
                        push(g_q, lambda k=k: emit_u(b, c, k))

                # front-loaded DMAs: first x chunk, weights+masks, rest of b0
                load_chunk(0, 0)
                for w in ("wv", "wq", "wu"):
                    nc.sync.dma_start(out=wt[w], in_=wd[w])
                nc.sync.dma_start(out=mkt, in_=mkd)
                load_chunk(0, 1)
                nc.sync.dma_start(out=wot, in_=wod)
                nc.sync.dma_start(out=idt, in_=idd)
                load_chunk(0, 2)
                load_chunk(0, 3)
                for b in range(B):
                    for c in range(NIC):
                        push_chunk(b, c)

                for b in range(B):
                    for c in range(NIC):
                        drain((b, c))
                        gs = attention(b, c)
                        stg = stp.tile([128, 4, D], BF16, tag="st", name="stg")
                        for k in range(3, -1, -1):
                            push_front(
                                ("wo", b, c),
                                lambda b=b, c=c, g=gs[k], stg=stg, k=k: emit_wo(
                                    b, c, g, stg, k
                                ),
                            )
                while filler_q:
                    pop_one()

    nc.compile()
    return nc


def _host_inputs(x, token_types, seq_lens, W_q, W_k, W_v, W_u, W_o, sched, masks_np):
    x = np.asarray(x, dtype=np.float32)
    W = {
        "wq": np.asarray(W_q, np.float32) / 8.0,
        "wk": np.asarray(W_k, np.float32),
        "wv": np.asarray(W_v, np.float32),
        "wu": np.asarray(W_u, np.float32),
    }
    Wo = np.asarray(W_o, np.float32)
    shared = {"ident": np.eye(128, dtype=BF), "masks": masks_np}
    for b in range(B):
        xb = x[b].astype(BF)[sched[b]["perm"]]  # sorted tokens [L, D]
        shared[f"x{b}"] = np.ascontiguousarray(
            xb.reshape(NIC, 512, NDC, 128).transpose(0, 3, 2, 1)
        )  # [c, p, dc, l']
    in_maps = []
    for core in range(8):
        e0 = E * core
        im = dict(shared)
        for w, Wm in W.items():
            im[w] = np.ascontiguousarray(
                Wm[e0 : e0 + E].astype(BF).reshape(E, NDC, 128).transpose(2, 1, 0)
            )  # [p, dc, e]
        im["wo"] = np.ascontiguousarray(Wo[:, e0 : e0 + E].astype(BF).T)  # [p, d]
        in_maps.append(im)
    return in_maps


def kernel(x, token_types, seq_lens, W_q, W_k, W_v, W_u, W_o, **_run_kwargs):
    sig, sched, masks_np = _mk_schedule(np.asarray(token_types), np.asarray(seq_lens))
    if _cache.get("sig") != sig:
        _cache["nc"] = _build(sched, masks_np.shape[1])
        _cache["sig"] = sig
    nc = _cache["nc"]
    in_maps = _host_inputs(
        x, token_types, seq_lens, W_q, W_k, W_v, W_u, W_o, sched, masks_np
    )
    try:
        res = run_bass_kernel_spmd(nc, in_maps, list(range(8)), **_run_kwargs)
    except Exception as ex:
        if "UNRECOVERABLE" not in str(ex) and "UNAVAILABLE" not in str(ex):
            raise
        res = run_bass_kernel_spmd(nc, in_maps, list(range(8)), **_run_kwargs)
    _cache["last_result"] = res
    full = np.zeros((B, L, D), np.float64)
    for core in range(8):
        o = res.results[core]["outp"].astype(np.float64)  # [b, c, p, k, d]
        full += o.transpose(0, 1, 3, 2, 4).reshape(B, L, D)
    out = np.empty_like(full)
    for b in range(B):
        out[b, sched[b]["perm"]] = full[b]
    return out.astype(np.float32)
